# revision 1
# baseline (speedup 1.0000x reference)
"""Masked dot-product attention (B=8, Lq=Lk=2048, D=64) on 8 Trainium2 NeuronCores.

Strategy
--------
Only keys k < valid_len[b] contribute (exp(-1e6) underflows to exactly 0), and
scores are ~N(0,1) so softmax needs no max-subtraction; unnormalized partial
sums over key-chunks are purely additive.  We therefore split work at
(batch, 128-key-chunk) granularity and load-balance those units across the 8
cores, combining partials on the host.

Per work unit (batch b, key chunk c), a core computes (layouts transposed so
no on-chip transposes are ever needed):
    S^T[k, q] = K_c^T Q^T          (PE, fp32r, contraction d=64)
    E = exp(S^T/8 + mask_bias)     (ACT, fused scale+mask+exp, bias per k-row)
    O^T[d', q] += V'_c^T E         (PE, fp32r, contraction k=128)
where V' = [V_c | 1] so row 64 of O^T accumulates the softmax denominator.

Each core has up to 3 "slots" (distinct Q^T buffers); units sharing a slot
share a batch and accumulate into the slot's PSUM output on-chip.  The host
assigns (batch, chunk) units to (core, slot) bins -- program structure (slot
capacities) is specialized to the actual valid_len at build time -- then sums
per-batch partials and divides by the denominator.
"""

import sys
import math

sys.path.insert(0, "/opt/trn_rl_repo")

import numpy as np

import concourse.bass as bass
import concourse.bacc as bacc
import concourse.mybir as mybir
import concourse.tile as tile
from concourse.bass_utils import run_bass_kernel_spmd

F32 = mybir.dt.float32
F32R = mybir.dt.float32r

B, L, D = 8, 2048, 64
NCORES = 8
CHUNK = 128          # key rows per work unit
NEG = -1e6
SCALE = 1.0 / 8.0    # 1/sqrt(64)
QH = 1024            # q processed in halves for PSUM budget


# --------------------------------------------------------------------------
# host-side scheduling: assign (batch, chunk) units to (core, slot) bins
# --------------------------------------------------------------------------

def _greedy_assign(chunks, caps):
    """Assign each batch's chunks to bins of 8 cores x caps; each bin holds
    chunks of a single batch.  Returns {(core, slot): (batch, [chunk_ids])}
    or None if infeasible."""
    bins = []  # (cap, core, slot)
    for core in range(NCORES):
        for s, c in enumerate(caps):
            bins.append([c, core, s])
    # big batches first; take largest free bins first
    order = sorted(range(len(chunks)), key=lambda b: -chunks[b])
    free = sorted(bins, key=lambda x: -x[0])
    assign = {}
    for b in order:
        rem = chunks[b]
        next_chunk = 0
        while rem > 0:
            if not free:
                return None
            # largest free bin; prefer smallest bin that still fits all of rem
            pick = None
            for i in range(len(free) - 1, -1, -1):
                if free[i][0] >= rem:
                    pick = i
                    break
            if pick is None:
                pick = 0  # largest
            cap, core, s = free.pop(pick)
            take = min(cap, rem)
            assign[(core, s)] = (b, list(range(next_chunk, next_chunk + take)))
            next_chunk += take
            rem -= take
    return assign


def _schedule(chunks):
    """Pick slot capacities (shared program structure) + assignment."""
    total = sum(chunks)
    lo = max(1, math.ceil(total / NCORES))
    for U in range(lo, 17):
        caps_opts = []
        for c0 in range(U, 0, -1):
            for c1 in range(min(c0, U - c0), -1, -1):
                c2 = U - c0 - c1
                if c2 < 0 or c2 > c1:
                    continue
                caps = tuple(c for c in (c0, c1, c2) if c > 0)
                caps_opts.append(caps)
        # prefer more-balanced splits first (better head-of-line pipelining)
        caps_opts.sort(key=lambda cs: (len(cs), max(cs)))
        for caps in caps_opts:
            asg = _greedy_assign(chunks, caps)
            if asg is not None:
                return caps, asg
    caps = (16,)
    asg = {(b, 0): (b, list(range(chunks[b]))) for b in range(B)}
    return caps, asg


# --------------------------------------------------------------------------
# device program (one NEFF shared by all 8 cores; structure = caps)
# --------------------------------------------------------------------------

def _build_program(caps):
    S = len(caps)
    U = sum(caps)
    nc = bacc.Bacc("TRN2", target_bir_lowering=False)

    qts_d = nc.dram_tensor("qts", [S, D, L], F32R, kind="ExternalInput")
    ktp_d = nc.dram_tensor("ktp", [U, D, CHUNK], F32R, kind="ExternalInput")
    vp_d = nc.dram_tensor("vp", [U, CHUNK, D + 1], F32R, kind="ExternalInput")
    mb_d = nc.dram_tensor("mb", [CHUNK, U], F32, kind="ExternalInput")
    out_d = nc.dram_tensor("out", [S, D + 1, L], F32, kind="ExternalOutput")

    slot_units = []
    u0 = 0
    for c in caps:
        slot_units.append(list(range(u0, u0 + c)))
        u0 += c

    with tile.TileContext(nc) as tc:
        with (
            tc.tile_pool(name="const", bufs=1) as const,
            tc.tile_pool(name="psS", bufs=2, space="PSUM") as psS_pool,
            tc.tile_pool(name="psO", bufs=2, space="PSUM") as psO_pool,
            tc.tile_pool(name="epool", bufs=3) as epool,
            tc.tile_pool(name="stage", bufs=2) as stage_pool,
        ):
            qts_sb = const.tile([D, S, L], F32R, tag="qts")
            ktp_sb = const.tile([D, U, CHUNK], F32R, tag="ktp")
            vp_sb = const.tile([CHUNK, U, D + 1], F32R, tag="vp")
            mb_sb = const.tile([CHUNK, U], F32, tag="mb")

            nc.sync.dma_start(mb_sb[:], mb_d[:, :])
            for s in range(S):
                us = slot_units[s]
                nc.sync.dma_start(qts_sb[:, s, :], qts_d[s, :, :])
                nc.sync.dma_start(
                    ktp_sb[:, us[0] : us[-1] + 1, :],
                    ktp_d[us[0] : us[-1] + 1, :, :].rearrange("u d k -> d u k"),
                )
                nc.sync.dma_start(
                    vp_sb[:, us[0] : us[-1] + 1, :],
                    vp_d[us[0] : us[-1] + 1, :, :].rearrange("u k d -> k u d"),
                )

            for s in range(S):
                for h in range(2):
                    psO = psO_pool.tile([D + 1, QH], F32, tag="psO")
                    cap = caps[s]
                    for i in range(cap):
                        u = slot_units[s][i]
                        psS = psS_pool.tile([CHUNK, QH], F32, tag="psS")
                        for j in range(QH // 512):
                            nc.tensor.matmul(
                                psS[:, j * 512 : (j + 1) * 512],
                                ktp_sb[:, u, :],
                                qts_sb[:, s, h * QH + j * 512 : h * QH + (j + 1) * 512],
                                start=True,
                                stop=True,
                            )
                        e_sb = epool.tile([CHUNK, QH], F32R, tag="e")
                        nc.scalar.activation(
                            e_sb[:],
                            psS[:],
                            mybir.ActivationFunctionType.Exp,
                            bias=mb_sb[:, u : u + 1],
                            scale=SCALE,
                        )
                        for j in range(QH // 512):
                            nc.tensor.matmul(
                                psO[:, j * 512 : (j + 1) * 512],
                                vp_sb[:, u, :],
                                e_sb[:, j * 512 : (j + 1) * 512],
                                start=(i == 0),
                                stop=(i == cap - 1),
                            )
                    stage = stage_pool.tile([D + 1, QH], F32, tag="stage")
                    nc.vector.tensor_copy(stage[:], psO[:])
                    nc.sync.dma_start(out_d[s, :, h * QH : (h + 1) * QH], stage[:])
    nc.compile()
    return nc


# --------------------------------------------------------------------------
# host packing + gather
# --------------------------------------------------------------------------

def _pack_inputs(Q, K, V, valid_len, caps, asg):
    S = len(caps)
    U = sum(caps)
    slot_u0 = np.cumsum([0] + list(caps))[:-1]

    QT = np.ascontiguousarray(Q.transpose(0, 2, 1))  # [B, D, L]
    KT = np.ascontiguousarray(K.transpose(0, 2, 1))  # [B, D, L]

    in_maps = []
    for core in range(NCORES):
        qts = np.zeros((S, D, L), np.float32)
        ktp = np.zeros((U, D, CHUNK), np.float32)
        vp = np.zeros((U, CHUNK, D + 1), np.float32)
        mb = np.full((CHUNK, U), NEG, np.float32)
        for s in range(S):
            ent = asg.get((core, s))
            if ent is None:
                continue
            b, chunk_ids = ent
            qts[s] = QT[b]
            for i, c in enumerate(chunk_ids):
                u = slot_u0[s] + i
                k0 = c * CHUNK
                ktp[u] = KT[b][:, k0 : k0 + CHUNK]
                vp[u, :, :D] = V[b][k0 : k0 + CHUNK]
                nvalid = int(min(max(valid_len[b] - k0, 0), CHUNK))
                vp[u, :nvalid, D] = 1.0
                mb[:nvalid, u] = 0.0
        in_maps.append({"qts": qts, "ktp": ktp, "vp": vp, "mb": mb})
    return in_maps


def _gather(results, caps, asg):
    acc = np.zeros((B, D + 1, L), np.float64)
    for core in range(NCORES):
        out = results[core]["out"]  # [S, D+1, L]
        for s in range(len(caps)):
            ent = asg.get((core, s))
            if ent is None:
                continue
            b, _ = ent
            acc[b] += out[s]
    out = acc[:, :D, :] / acc[:, D : D + 1, :]
    return np.ascontiguousarray(out.transpose(0, 2, 1)).astype(np.float32)


_PROGRAM_CACHE = {}


def kernel(Q, K, V, valid_len, **kw):
    Q = np.asarray(Q, dtype=np.float32)
    K = np.asarray(K, dtype=np.float32)
    V = np.asarray(V, dtype=np.float32)
    vl = np.asarray(valid_len).astype(np.int64)

    chunks = [int(math.ceil(max(int(v), 1) / CHUNK)) for v in vl]
    caps, asg = _schedule(chunks)

    if caps not in _PROGRAM_CACHE:
        _PROGRAM_CACHE[caps] = _build_program(caps)
    nc = _PROGRAM_CACHE[caps]

    in_maps = _pack_inputs(Q, K, V, vl, caps, asg)
    res = run_bass_kernel_spmd(nc, in_maps, core_ids=list(range(NCORES)))
    return _gather(res.results, caps, asg)
